# revision 1
# baseline (speedup 1.0000x reference)
"""DINOv2 LoRA featurizer histogram-binning kernel for TRN2 (8 NeuronCores).

Reference computation (per sample):
  x: [37, 37, 384] -> bx = x^T [384, 37, 37]
  pool0 = bx, pool1 = AvgPool2d(3, stride 1, pad 1, count_include_pad=False)
  17 bins = border-clamped shifts of pool0 (9, offsets +-1) and pool1 (8, offsets +-3)
  out = concat(bins, 12 zero bins) -> [29*384, 37, 37]

Sharding: pure data parallel, sample b -> core b.
"""

import numpy as np

B = 8
W = 37          # spatial side
WW = W * W      # 1369
D = 384
P = 128
ST = D // P     # 3 channel tiles of 128
NBINS = 29
PAD0, PAD1 = 1, 3
W0P, W1P = W + 2 * PAD0, W + 2 * PAD1   # 39, 43

# bin offset order exactly as the reference nested loops
OFFSETS = [(0, dy, dx) for dy in (-1, 0, 1) for dx in (-1, 0, 1)]
OFFSETS += [(1, dy, dx) for dy in (-3, 0, 3) for dx in (-3, 0, 3) if (dy, dx) != (0, 0)]

_CACHE = {}


def _inv_count() -> np.ndarray:
    """1 / (3x3 valid-neighbor count) per spatial position, tiled to [P, WW]."""
    ones = np.ones((W, W), np.float64)
    cnt = np.zeros((W, W), np.float64)
    for dy in (-1, 0, 1):
        for dx in (-1, 0, 1):
            ys = slice(max(0, -dy), W - max(0, dy))
            cnt[max(0, dy) : W - max(0, -dy), max(0, dx) : W - max(0, -dx)] += ones[
                ys, slice(max(0, -dx), W - max(0, dx))
            ]
    inv = (1.0 / cnt).astype(np.float32).reshape(WW)
    return np.broadcast_to(inv, (P, WW)).copy()


def _build_nc():
    import concourse.bass as bass  # noqa: F401
    import concourse.tile as tile
    from concourse import bacc, mybir
    from contextlib import ExitStack

    f32 = mybir.dt.float32
    nc = bacc.Bacc("TRN2", target_bir_lowering=False, debug=False)

    xt = nc.declare_dram_parameter("xt", [ST, P, WW], f32, isOutput=False)
    invcnt = nc.declare_dram_parameter("invcnt", [P, WW], f32, isOutput=False)
    out = nc.declare_dram_parameter("out", [NBINS, ST, P, WW], f32, isOutput=True)

    with tile.TileContext(nc) as tc, ExitStack() as ctx:
        perm = ctx.enter_context(tc.tile_pool(name="perm", bufs=1))
        tmp = ctx.enter_context(tc.tile_pool(name="tmp", bufs=2))
        stage_pool = ctx.enter_context(tc.tile_pool(name="stage", bufs=4))

        inv = perm.tile([P, WW], f32, name="inv")
        nc.sync.dma_start(inv[:, :], invcnt.ap())

        p0 = [perm.tile([P, W0P, W0P], f32, name=f"p0_{t}") for t in range(ST)]
        p1 = [perm.tile([P, W1P, W1P], f32, name=f"p1_{t}") for t in range(ST)]

        for t in range(ST):
            X = tmp.tile([P, WW], f32, name="X", tag="X")
            nc.sync.dma_start(X[:, :], xt.ap()[t])
            X3 = X.rearrange("p (a b) -> p a b", a=W, b=W)

            # P0 = x, replicate-padded by 1
            q = p0[t]
            nc.vector.tensor_copy(q[:, 1 : 1 + W, 1 : 1 + W], X3[:, :, :])
            nc.scalar.copy(q[:, 0, 1 : 1 + W], X3[:, 0, :])
            nc.scalar.copy(q[:, 1 + W, 1 : 1 + W], X3[:, W - 1, :])
            nc.scalar.copy(q[:, :, 0], q[:, :, 1])
            nc.scalar.copy(q[:, :, 1 + W], q[:, :, W])

            # separable 3x3 sum with zero boundary: col pass into T, row pass into S
            T = tmp.tile([P, WW], f32, name="T", tag="T")
            T3 = T.rearrange("p (a b) -> p a b", a=W, b=W)
            nc.vector.tensor_add(T3[:, :, 0 : W - 1], X3[:, :, 0 : W - 1], X3[:, :, 1:W])
            nc.vector.tensor_copy(T3[:, :, W - 1], X3[:, :, W - 1])
            nc.vector.tensor_add(T3[:, :, 1:W], T3[:, :, 1:W], X3[:, :, 0 : W - 1])

            S = tmp.tile([P, WW], f32, name="S", tag="S")
            S3 = S.rearrange("p (a b) -> p a b", a=W, b=W)
            nc.vector.tensor_add(S3[:, 0 : W - 1, :], T3[:, 0 : W - 1, :], T3[:, 1:W, :])
            nc.vector.tensor_copy(S3[:, W - 1, :], T3[:, W - 1, :])
            nc.vector.tensor_add(S3[:, 1:W, :], S3[:, 1:W, :], T3[:, 0 : W - 1, :])

            # P1 = S * invcnt, written into the center of the 3-padded tile
            r = p1[t]
            nc.vector.tensor_mul(
                r[:, PAD1 : PAD1 + W, PAD1 : PAD1 + W],
                S3[:, :, :],
                inv.rearrange("p (a b) -> p a b", a=W, b=W),
            )
            # replicate borders (rows first, then full-height cols)
            for i in range(PAD1):
                nc.scalar.copy(r[:, i, PAD1 : PAD1 + W], r[:, PAD1, PAD1 : PAD1 + W])
                nc.scalar.copy(
                    r[:, PAD1 + W + i, PAD1 : PAD1 + W], r[:, PAD1 + W - 1, PAD1 : PAD1 + W]
                )
            for i in range(PAD1):
                nc.scalar.copy(r[:, :, i], r[:, :, PAD1])
                nc.scalar.copy(r[:, :, PAD1 + W + i], r[:, :, PAD1 + W - 1])

        # 17 data bins: window-copy from padded pools into staging, DMA out.
        # Bins 17..28 are zeros: ExternalOutput buffers are pre-zeroed by the
        # runner (native path zero-allocates; PJRT path donates zero buffers),
        # so we never write them.
        engines = [nc.vector, nc.scalar, nc.gpsimd]
        n_copy = 0
        for p, (k, dy, dx) in enumerate(OFFSETS):
            pad = PAD0 if k == 0 else PAD1
            src = p0 if k == 0 else p1
            stage = stage_pool.tile([P, ST, W, W], f32, name=f"stage{p}", tag="stage")
            for t in range(ST):
                win = src[t][:, pad + dy : pad + dy + W, pad + dx : pad + dx + W]
                eng = engines[n_copy % len(engines)]
                n_copy += 1
                if eng is nc.scalar:
                    nc.scalar.copy(stage[:, t], win)
                else:
                    eng.tensor_copy(stage[:, t], win)
            dst = out.ap()[p].transpose([1, 0, 2])  # [ST,P,WW] -> [P,ST,WW]
            nc.sync.dma_start(dst, stage.rearrange("p t a b -> p t (a b)"))

    nc.compile()
    return nc


def get_nc():
    if "nc" not in _CACHE:
        _CACHE["nc"] = _build_nc()
    return _CACHE["nc"]


def make_in_maps(x: np.ndarray):
    x = np.ascontiguousarray(x, dtype=np.float32)
    assert x.shape == (B, W, W, D), x.shape
    inv = _inv_count()
    maps = []
    for b in range(B):
        xt = x[b].transpose(2, 0, 1).reshape(ST, P, WW)
        maps.append({"xt": np.ascontiguousarray(xt), "invcnt": inv})
    return maps


def run(x: np.ndarray, **kw):
    from concourse.bass_utils import run_bass_kernel_spmd

    nc = get_nc()
    res = run_bass_kernel_spmd(nc, make_in_maps(x), core_ids=list(range(B)), **kw)
    outs = np.stack([res.results[b]["out"].reshape(NBINS * D, W, W) for b in range(B)])
    return outs, res


def kernel(x: np.ndarray) -> np.ndarray:
    outs, _ = run(x)
    return outs
